# revision 17
# baseline (speedup 1.0000x reference)
"""HGConv fused kernel for one TRN2 chip (8 NeuronCores), SPMD via Bass/Tile.

Hardcoded for M=16384 nodes, E=4096 hyperedges, D=300, N_CAT=3, 8 cores.

Edge-sharded design (v3) — no mid-kernel collectives:
  - Core c owns edges Ec = [512c, 512(c+1)).  Inputs per core: full X and
    inc[:, Ec] in bf16, HOST-PREPACKED into partition-major layout
    [128, t*d] so every DMA line is a multi-KB contiguous row;
    edge_feats[Ec].T (f32, host-transposed); small weights.
  - Phase 1 computes IX_c = inc_c.T @ X over ALL 16384 nodes, m-major
    (x and inc tiles stream through small pools; 4 psum banks accumulate
    the 4 x 128-edge sub-blocks).
  - Tail (batched, stage-major): IX -> transpose -> edge_att = IX@W_att
    (f32r), rowwise softmax over d, ef = attn*IX, transpose, ef2T =
    alpha*eftT + (1-alpha)*(W_proj.T @ efT), scores = ec_W_att.T @ ef2T
    (|scores| < 5 so exp is unstabilized), G2 = ef2 @ (ec_W_proj@fc_W),
    p2 = expw.T @ G2, z = sum(expw).
  - One 4-float AllGather; every core combines the 8 partials with a
    ones-vector matmul and emits logits = p2/z + (ecb@fcW + fcb).
"""

import sys

for _p in ("/opt/trn_rl_repo", "/opt/pypackages"):
    if _p not in sys.path:
        sys.path.append(_p)

import numpy as np

import concourse.bacc as bacc
import concourse.tile as tile
from concourse import masks, mybir
from concourse.bass_utils import run_bass_kernel_spmd

F32 = mybir.dt.float32
F32R = mybir.dt.float32r
BF16 = mybir.dt.bfloat16
AX = mybir.AxisListType
OP = mybir.AluOpType
AF = mybir.ActivationFunctionType

NCORES = 8
M, E, D, NCAT = 16384, 4096, 300, 3
E_SH = E // NCORES          # 512 edges per core
ET = E_SH // 128            # 4 e-sub-blocks per core
MT = M // 128               # 128 m-tiles
MG = 4                      # m-tiles per streamed group
W_ROW = D + E_SH            # 812 bf16 per m-row in the packed layout
NG = MT // MG               # 8 groups
DCH = (128, 128, 44)        # d split into partition chunks
DOF = (0, 128, 256)


def _build(alpha: float):
    nc = bacc.Bacc("TRN2", target_bir_lowering=False, debug=False,
                   num_devices=NCORES)
    # prepacked [128, t*(d+e)] partition-major interleaved x|inc rows
    xi_d = nc.dram_tensor("xi", [128, MT * (D + E_SH)], BF16,
                          kind="ExternalInput")
    eft_d = nc.dram_tensor("eft", [D, E_SH], F32, kind="ExternalInput")
    watt_d = nc.dram_tensor("watt", [D, D], F32, kind="ExternalInput")
    wproj_d = nc.dram_tensor("wproj", [D, D], F32, kind="ExternalInput")
    ecwatt_d = nc.dram_tensor("ecwatt", [D, 1], F32, kind="ExternalInput")
    # ec_W_proj passed TRANSPOSED from host (only used via W2 = ecp @ fcw)
    ecpT_d = nc.dram_tensor("ecpt", [D, D], F32, kind="ExternalInput")
    ecb_d = nc.dram_tensor("ecb", [D], F32, kind="ExternalInput")
    fcw_d = nc.dram_tensor("fcw", [D, NCAT], F32, kind="ExternalInput")
    fcb_d = nc.dram_tensor("fcb", [NCAT], F32, kind="ExternalInput")
    out_d = nc.dram_tensor("out", [1, NCAT], F32, kind="ExternalOutput")

    groups = [list(range(NCORES))]

    def r(ap):
        return ap.bitcast(F32R)

    with tile.TileContext(nc) as tc, \
         tc.tile_pool(name="sb", bufs=1) as sb, \
         tc.tile_pool(name="dram", bufs=1, space="DRAM") as dram:

        prt_d = dram.tile([4], F32)            # AllGather input  [p2, z]
        gat_d = dram.tile([NCORES * 4], F32)   # AllGather output
        wrm_d = dram.tile([4], F32)            # warm-up collective in
        wgt_d = dram.tile([NCORES * 4], F32)   # warm-up collective out

        # ---------- small-weight tiles (loads issued mid-phase-1) ----------
        watt_sb = sb.tile([128, 3, D], F32)
        wproj_sb = sb.tile([128, 3, D], F32)
        ecpT_sb = sb.tile([128, 3, D], F32)
        fcw_sb = sb.tile([128, 3, NCAT], F32)
        ecwatt_sb = sb.tile([128, 3, 1], F32)
        ecbc_sb = sb.tile([128, 3, 1], F32)
        eft_sb = sb.tile([128, 3, E_SH], F32)
        efs_sb = sb.tile([128, 3, E_SH], F32)
        fcb_sb = sb.tile([1, NCAT], F32)
        ident = sb.tile([128, 128], F32)
        masks.make_identity(nc, ident[:])
        ones8_sb = sb.tile([NCORES, 1], F32)
        nc.vector.memset(ones8_sb[:], 1.0)

        def load_weights(part):
            # on the sync ring AFTER the first phase-1 groups, in small
            # slices so the ring sequencer never starves the input stream
            i, (c, o) = part, (DCH[part % 3], DOF[part % 3])
            if part < 3:
                nc.sync.dma_start(watt_sb[:c, part, :].bitcast(F32R),
                                  watt_d[o:o + c, :].bitcast(F32R))
                nc.sync.dma_start(wproj_sb[:c, part, :].bitcast(F32R),
                                  wproj_d[o:o + c, :].bitcast(F32R))
                nc.sync.dma_start(eft_sb[:c, part, :], eft_d[o:o + c, :])
                nc.scalar.mul(efs_sb[:c, part, :], eft_sb[:c, part, :],
                              float(alpha))
            elif part == 3:
                for i, (c, o) in enumerate(zip(DCH, DOF)):
                    nc.sync.dma_start(ecpT_sb[:c, i, :], ecpT_d[o:o + c, :])
                    nc.sync.dma_start(fcw_sb[:c, i, :], fcw_d[o:o + c, :])
            else:
                for i, (c, o) in enumerate(zip(DCH, DOF)):
                    nc.sync.dma_start(ecwatt_sb[:c, i, :].bitcast(F32R),
                                      ecwatt_d[o:o + c, :].bitcast(F32R))
                    nc.sync.dma_start(
                        ecbc_sb[:c, i, 0:1],
                        ecb_d[o:o + c].rearrange("(p o) -> p o", o=1))
                nc.sync.dma_start(fcb_sb[:],
                                  fcb_d.ap().rearrange("(o d) -> o d", o=1))

        # persistent tail state
        ix_sb = sb.tile([128, ET, D], F32)
        ex_sb = sb.tile([128, ET, D], F32)
        ef_sb = sb.tile([128, ET, D], F32)
        ixT_sb = sb.tile([128, 3, E_SH], F32)
        efT_sb = sb.tile([128, 3, E_SH], F32)
        ef2T_sb = sb.tile([128, 3, E_SH], F32)
        w2_sb = sb.tile([128, 3, NCAT], F32)
        stat_sb = sb.tile([128, ET, 4], F32)
        expw_sb = sb.tile([1, E_SH + 4], F32)
        expcol_sb = sb.tile([128, ET], F32)
        g2_sb = sb.tile([128, ET, NCAT], F32)
        b2_sb = sb.tile([1, 4], F32)
        prt_sb = sb.tile([1, 4], F32)
        g8_sb = sb.tile([NCORES, 4], F32)
        cmb_sb = sb.tile([1, 4], F32)
        logit_sb = sb.tile([1, NCAT], F32)

        # ---------- phase 1: m-major streamed IX = inc.T @ X ----------
        with tc.tile_pool(name="xipool", bufs=16) as xipool, \
             tc.tile_pool(name="pp1", bufs=4, space="PSUM") as pp1, \
             tc.tile_pool(name="ppw", bufs=1, space="PSUM") as ppw:

            accs = [pp1.tile([128, D], F32, tag="p1", name=f"acc{es}")
                    for es in range(ET)]

            for g in range(NG):
                xi = xipool.tile([128, MG * W_ROW], BF16, tag="xi",
                                 name=f"xi{g}")
                nc.sync.dma_start(
                    xi[:], xi_d[:, MG * W_ROW * g:MG * W_ROW * (g + 1)])
                for mt in range(MG):
                    for es in range(ET):
                        nc.tensor.matmul(
                            accs[es][:],
                            xi[:, mt * W_ROW + D + 128 * es:
                               mt * W_ROW + D + 128 * (es + 1)],
                            xi[:, mt * W_ROW:mt * W_ROW + D],
                            start=(g == 0 and mt == 0),
                            stop=(g == NG - 1 and mt == MG - 1))
                if g == 0 or g == 12:
                    # warm the collective path early: absorbs CC-engine
                    # cold start + inter-core launch skew while PE works
                    nc.gpsimd.collective_compute(
                        "AllGather", OP.bypass, replica_groups=groups,
                        ins=[wrm_d.opt()], outs=[wgt_d.opt()])
                if 6 <= g <= 14 and g % 2 == 0:
                    load_weights((g - 6) // 2)
                if g == 18:
                    # device precompute, hidden under phase 1:
                    # W2 = ec_W_proj @ fc_W ; b2 = ecb @ fcW + fcb
                    for j, (cj, oj) in enumerate(zip(DCH, DOF)):
                        w2p = ppw.tile([128, NCAT], F32, tag="w",
                                       name=f"w2p{j}")
                        for i, (ci, _) in enumerate(zip(DCH, DOF)):
                            nc.tensor.matmul(w2p[:cj, :],
                                             ecpT_sb[:ci, i, oj:oj + cj],
                                             fcw_sb[:ci, i, :],
                                             start=(i == 0), stop=(i == 2))
                        nc.scalar.copy(w2_sb[:cj, j, :], w2p[:cj, :])
                    b2p = ppw.tile([1, NCAT], F32, tag="w", name="b2p")
                    for i, (ci, _) in enumerate(zip(DCH, DOF)):
                        nc.tensor.matmul(b2p[:], ecbc_sb[:ci, i, :],
                                         fcw_sb[:ci, i, :],
                                         start=(i == 0), stop=(i == 2))
                    nc.vector.tensor_add(b2_sb[:, 0:NCAT], b2p[:],
                                         fcb_sb[:])

            # psum -> sbuf (inside pp1 scope)
            for es in range(ET):
                nc.vector.tensor_copy(ix_sb[:, es, :], accs[es][:])

        # ---------- batched tail ----------
        with tc.tile_pool(name="ppt", bufs=3, space="PSUM") as ppt, \
             tc.tile_pool(name="ppa", bufs=2, space="PSUM") as ppa, \
             tc.tile_pool(name="ppj", bufs=2, space="PSUM") as ppj, \
             tc.tile_pool(name="pps", bufs=1, space="PSUM") as pps:

            def ef_transpose(es):
                for i, (c, o) in enumerate(zip(DCH, DOF)):
                    tp = ppt.tile([128, 128], F32, tag="tp",
                                  name=f"tpe_{es}_{i}")
                    nc.tensor.transpose(tp[:c, :], ef_sb[:, es, o:o + c],
                                        ident[:])
                    nc.scalar.copy(
                        efT_sb[:c, i,
                               128 * es:128 * (es + 1)].bitcast(F32R),
                        tp[:c, :])

            # stage 1+2 per es: IX -> ixT ; att (f32r) ; softmax ; ef
            for es in range(ET):
                for i, (c, o) in enumerate(zip(DCH, DOF)):
                    tp = ppt.tile([128, 128], F32, tag="tp",
                                  name=f"tpa_{es}_{i}")
                    nc.tensor.transpose(tp[:c, :], ix_sb[:, es, o:o + c],
                                        ident[:])
                    nc.scalar.copy(
                        ixT_sb[:c, i,
                               128 * es:128 * (es + 1)].bitcast(F32R),
                        tp[:c, :])
                att = ppa.tile([128, D], F32, tag="att", name=f"att{es}")
                for i, (c, _) in enumerate(zip(DCH, DOF)):
                    nc.tensor.matmul(att[:],
                                     r(ixT_sb[:c, i,
                                              128 * es:128 * (es + 1)]),
                                     r(watt_sb[:c, i, :]),
                                     start=(i == 0), stop=(i == 2))
                nmax = stat_sb[:, es, 0:1]
                nc.vector.tensor_reduce(nmax, att[:], axis=AX.X, op=OP.max,
                                        negate=True)
                rsum = stat_sb[:, es, 1:2]
                nc.scalar.activation(ex_sb[:, es, :], att[:], AF.Exp,
                                     bias=nmax, scale=1.0, accum_out=rsum)
                rcp = stat_sb[:, es, 2:3]
                nc.vector.reciprocal(rcp, rsum)
                nc.vector.scalar_tensor_tensor(
                    ef_sb[:, es, :], ex_sb[:, es, :], rcp, ix_sb[:, es, :],
                    op0=OP.mult, op1=OP.mult)
                if es >= 1:
                    ef_transpose(es - 1)
            ef_transpose(ET - 1)

            # stage 4: ef2T = alpha*eftT + (1-alpha) * (W_proj.T @ efT)
            for j, (cj, oj) in enumerate(zip(DCH, DOF)):
                prj = ppj.tile([128, E_SH], F32, tag="prj", name=f"prj{j}")
                for i, (ci, _) in enumerate(zip(DCH, DOF)):
                    nc.tensor.matmul(prj[:cj, :],
                                     r(wproj_sb[:ci, i, oj:oj + cj]),
                                     r(efT_sb[:ci, i, :]),
                                     start=(i == 0), stop=(i == 2))
                nc.vector.scalar_tensor_tensor(
                    ef2T_sb[:cj, j, :].bitcast(F32R), prj[:cj, :],
                    float(1.0 - alpha), efs_sb[:cj, j, :], op0=OP.mult,
                    op1=OP.add)

            # stage 5: scores first on PE, then G2 while exp runs
            sc = ppj.tile([1, E_SH], F32, tag="prj", name="sc")
            for i, (ci, _) in enumerate(zip(DCH, DOF)):
                nc.tensor.matmul(sc[:], r(ecwatt_sb[:ci, i, :]),
                                 r(ef2T_sb[:ci, i, :]),
                                 start=(i == 0), stop=(i == 2))
            expw = expw_sb[:, 0:E_SH]
            z = expw_sb[:, E_SH:E_SH + 1]
            nc.scalar.activation(expw, sc[:], AF.Exp, bias=0.0, scale=1.0,
                                 accum_out=z)

            # stage 6: G2 = ef2 @ W2 (PE, overlaps exp) ; p2 = expw.T @ G2
            for es in range(ET):
                g2 = pps.tile([128, NCAT], F32, tag="small", name=f"g2_{es}")
                for i, (ci, _) in enumerate(zip(DCH, DOF)):
                    nc.tensor.matmul(g2[:],
                                     ef2T_sb[:ci, i,
                                             128 * es:128 * (es + 1)],
                                     w2_sb[:ci, i, :],
                                     start=(i == 0), stop=(i == 2))
                nc.scalar.copy(g2_sb[:, es, :], g2[:])
            for es in range(ET):
                tc1 = ppt.tile([128, 128], F32, tag="tp", name=f"tc1_{es}")
                nc.tensor.transpose(tc1[:, 0:1],
                                    expw[0:1, 128 * es:128 * (es + 1)],
                                    ident[0:1, 0:1])
                nc.scalar.copy(expcol_sb[:, es:es + 1], tc1[:, 0:1])
            p2 = pps.tile([1, NCAT], F32, tag="small", name="p2")
            for es in range(ET):
                nc.tensor.matmul(p2[:], expcol_sb[:, es:es + 1],
                                 g2_sb[:, es, :], start=(es == 0),
                                 stop=(es == ET - 1))

            nc.scalar.copy(prt_sb[:, 0:NCAT], p2[:])
            nc.scalar.copy(prt_sb[:, NCAT:NCAT + 1], z)
            nc.sync.dma_start(prt_d[:], prt_sb[0:1, :])

            # ---------- AllGather + tiny epilogue ----------
            nc.gpsimd.collective_compute(
                "AllGather", OP.bypass, replica_groups=groups,
                ins=[prt_d.opt()], outs=[gat_d.opt()])
            nc.sync.dma_start(g8_sb[:],
                              gat_d[:].rearrange("(c k) -> c k", c=NCORES))
            cmb = pps.tile([1, 4], F32, tag="small", name="cmb")
            nc.tensor.matmul(cmb[:], ones8_sb[:], g8_sb[:], start=True,
                             stop=True)
            rz = expw_sb[:, E_SH + 1:E_SH + 2]
            nc.vector.reciprocal(rz, cmb[:, NCAT:NCAT + 1])
            nc.vector.scalar_tensor_tensor(
                logit_sb[:], cmb[:, 0:NCAT], rz, b2_sb[:, 0:NCAT],
                op0=OP.mult, op1=OP.add)
            nc.sync.dma_start(out_d[:], logit_sb[:])

    nc.compile()
    return nc


_CACHE = {}


def get_nc(alpha: float):
    if alpha not in _CACHE:
        _CACHE[alpha] = _build(alpha)
    return _CACHE[alpha]


def _pack(a2d, rows, width):
    # (rows*128, width) row-major -> (128, rows*width) partition-major
    return np.ascontiguousarray(
        a2d.reshape(rows, 128, width).transpose(1, 0, 2).reshape(
            128, rows * width))


def make_in_maps(node_feats, edge_feats, inc_mat, W_att, W_proj,
                 ec_W_att, ec_W_proj, ec_b_proj, fc_W, fc_b):
    import ml_dtypes
    cc = lambda a: np.ascontiguousarray(np.asarray(a, np.float32))
    x_bf = np.asarray(node_feats, np.float32).astype(ml_dtypes.bfloat16)
    inc_f = np.asarray(inc_mat, np.float32)
    eft = np.asarray(edge_feats, np.float32).T  # (D, E)
    common = dict(watt=cc(W_att), wproj=cc(W_proj),
                  ecwatt=cc(ec_W_att).reshape(D, 1),
                  ecpt=cc(np.asarray(ec_W_proj, np.float32).T),
                  ecb=cc(ec_b_proj), fcw=cc(fc_W), fcb=cc(fc_b))
    in_maps = []
    for c in range(NCORES):
        sl = slice(E_SH * c, E_SH * (c + 1))
        inc_bf = inc_f[:, sl].astype(ml_dtypes.bfloat16)
        # interleave per m-row: [x_row (300) | inc_row (512)] bf16
        xi = np.empty((M, D + E_SH), dtype=ml_dtypes.bfloat16)
        xi[:, 0:D] = x_bf
        xi[:, D:] = inc_bf
        in_maps.append(dict(
            xi=_pack(xi, MT, D + E_SH),
            eft=np.ascontiguousarray(eft[:, sl]),
            **common))
    return in_maps


def kernel(node_feats, edge_feats, inc_mat, W_att, W_proj, alpha,
           ec_W_att, ec_W_proj, ec_b_proj, fc_W, fc_b, trace=False,
           mode=None):
    nc = get_nc(float(np.asarray(alpha)))
    in_maps = make_in_maps(node_feats, edge_feats, inc_mat, W_att, W_proj,
                           ec_W_att, ec_W_proj, ec_b_proj, fc_W, fc_b)
    res = run_bass_kernel_spmd(nc, in_maps, list(range(NCORES)), trace=trace)
    kernel.last_results = res
    return res.results[0]["out"].reshape(NCAT).astype(np.float32)


# revision 18
# speedup vs baseline: 1.1449x; 1.1449x over previous
"""HGConv fused kernel for one TRN2 chip (8 NeuronCores), SPMD via Bass/Tile.

Hardcoded for M=16384 nodes, E=4096 hyperedges, D=300, N_CAT=3, 8 cores.

Edge-sharded design (v3) — no mid-kernel collectives:
  - Core c owns edges Ec = [512c, 512(c+1)).  Inputs per core: full X and
    inc[:, Ec] in bf16, HOST-PREPACKED into partition-major layout
    [128, t*d] so every DMA line is a multi-KB contiguous row;
    edge_feats[Ec].T (f32, host-transposed); small weights.
  - Phase 1 computes IX_c = inc_c.T @ X over ALL 16384 nodes, m-major
    (x and inc tiles stream through small pools; 4 psum banks accumulate
    the 4 x 128-edge sub-blocks).
  - Tail (batched, stage-major): IX -> transpose -> edge_att = IX@W_att
    (f32r), rowwise softmax over d, ef = attn*IX, transpose, ef2T =
    alpha*eftT + (1-alpha)*(W_proj.T @ efT), scores = ec_W_att.T @ ef2T
    (|scores| < 5 so exp is unstabilized), G2 = ef2 @ (ec_W_proj@fc_W),
    p2 = expw.T @ G2, z = sum(expw).
  - One 4-float AllGather; every core combines the 8 partials with a
    ones-vector matmul and emits logits = p2/z + (ecb@fcW + fcb).
"""

import sys

for _p in ("/opt/trn_rl_repo", "/opt/pypackages"):
    if _p not in sys.path:
        sys.path.append(_p)

import numpy as np

import concourse.bacc as bacc
import concourse.tile as tile
from concourse import masks, mybir
from concourse.bass_utils import run_bass_kernel_spmd

F32 = mybir.dt.float32
F32R = mybir.dt.float32r
BF16 = mybir.dt.bfloat16
AX = mybir.AxisListType
OP = mybir.AluOpType
AF = mybir.ActivationFunctionType

NCORES = 8
M, E, D, NCAT = 16384, 4096, 300, 3
E_SH = E // NCORES          # 512 edges per core
ET = E_SH // 128            # 4 e-sub-blocks per core
MT = M // 128               # 128 m-tiles
MG = 4                      # m-tiles per streamed group
W_ROW = D + E_SH            # 812 bf16 per m-row in the packed layout
NG = MT // MG               # 8 groups
DCH = (128, 128, 44)        # d split into partition chunks
DOF = (0, 128, 256)


def _build(alpha: float):
    nc = bacc.Bacc("TRN2", target_bir_lowering=False, debug=False,
                   num_devices=NCORES)
    # prepacked [128, t*(d+e)] partition-major interleaved x|inc rows
    xi_d = nc.dram_tensor("xi", [128, MT * (D + E_SH)], BF16,
                          kind="ExternalInput")
    eft_d = nc.dram_tensor("eft", [D, E_SH], F32, kind="ExternalInput")
    watt_d = nc.dram_tensor("watt", [D, D], F32, kind="ExternalInput")
    wproj_d = nc.dram_tensor("wproj", [D, D], F32, kind="ExternalInput")
    ecwatt_d = nc.dram_tensor("ecwatt", [D, 1], F32, kind="ExternalInput")
    # ec_W_proj passed TRANSPOSED from host (only used via W2 = ecp @ fcw)
    ecpT_d = nc.dram_tensor("ecpt", [D, D], F32, kind="ExternalInput")
    ecb_d = nc.dram_tensor("ecb", [D], F32, kind="ExternalInput")
    fcw_d = nc.dram_tensor("fcw", [D, NCAT], F32, kind="ExternalInput")
    fcb_d = nc.dram_tensor("fcb", [NCAT], F32, kind="ExternalInput")
    out_d = nc.dram_tensor("out", [1, NCAT], F32, kind="ExternalOutput")

    groups = [list(range(NCORES))]

    def r(ap):
        return ap.bitcast(F32R)

    with tile.TileContext(nc) as tc, \
         tc.tile_pool(name="sb", bufs=1) as sb, \
         tc.tile_pool(name="dram", bufs=1, space="DRAM") as dram:

        prt_d = dram.tile([4], F32)            # AllGather input  [p2, z]
        gat_d = dram.tile([NCORES * 4], F32)   # AllGather output
        wrm_d = dram.tile([4], F32)            # warm-up collective in
        wgt_d = dram.tile([NCORES * 4], F32)   # warm-up collective out

        # ---------- small-weight tiles (loads issued mid-phase-1) ----------
        watt_sb = sb.tile([128, 3, D], F32)
        wproj_sb = sb.tile([128, 3, D], F32)
        ecpT_sb = sb.tile([128, 3, D], F32)
        fcw_sb = sb.tile([128, 3, NCAT], F32)
        ecwatt_sb = sb.tile([128, 3, 1], F32)
        ecbc_sb = sb.tile([128, 3, 1], F32)
        eft_sb = sb.tile([128, 3, E_SH], F32)
        efs_sb = sb.tile([128, 3, E_SH], F32)
        fcb_sb = sb.tile([1, NCAT], F32)
        ident = sb.tile([128, 128], F32)
        masks.make_identity(nc, ident[:])
        ones8_sb = sb.tile([NCORES, 1], F32)
        nc.vector.memset(ones8_sb[:], 1.0)

        def load_weights(part):
            # on the sync ring AFTER the first phase-1 groups, in small
            # slices so the ring sequencer never starves the input stream
            i, (c, o) = part, (DCH[part % 3], DOF[part % 3])
            if part < 3:
                nc.sync.dma_start(watt_sb[:c, part, :].bitcast(F32R),
                                  watt_d[o:o + c, :].bitcast(F32R))
                nc.sync.dma_start(wproj_sb[:c, part, :].bitcast(F32R),
                                  wproj_d[o:o + c, :].bitcast(F32R))
                nc.sync.dma_start(eft_sb[:c, part, :], eft_d[o:o + c, :])
                nc.scalar.mul(efs_sb[:c, part, :], eft_sb[:c, part, :],
                              float(alpha))
            elif part == 3:
                for i, (c, o) in enumerate(zip(DCH, DOF)):
                    nc.sync.dma_start(ecpT_sb[:c, i, :], ecpT_d[o:o + c, :])
                    nc.sync.dma_start(fcw_sb[:c, i, :], fcw_d[o:o + c, :])
            else:
                for i, (c, o) in enumerate(zip(DCH, DOF)):
                    nc.sync.dma_start(ecwatt_sb[:c, i, :].bitcast(F32R),
                                      ecwatt_d[o:o + c, :].bitcast(F32R))
                    nc.sync.dma_start(
                        ecbc_sb[:c, i, 0:1],
                        ecb_d[o:o + c].rearrange("(p o) -> p o", o=1))
                nc.sync.dma_start(fcb_sb[:],
                                  fcb_d.ap().rearrange("(o d) -> o d", o=1))

        # persistent tail state
        ix_sb = sb.tile([128, ET, D], F32)
        ex_sb = sb.tile([128, ET, D], F32)
        ef_sb = sb.tile([128, ET, D], F32)
        ixT_sb = sb.tile([128, 3, E_SH], F32)
        efT_sb = sb.tile([128, 3, E_SH], F32)
        ef2T_sb = sb.tile([128, 3, E_SH], F32)
        w2_sb = sb.tile([128, 3, NCAT], F32)
        stat_sb = sb.tile([128, ET, 4], F32)
        expw_sb = sb.tile([1, E_SH + 4], F32)
        expcol_sb = sb.tile([128, ET], F32)
        g2_sb = sb.tile([128, ET, NCAT], F32)
        b2_sb = sb.tile([1, 4], F32)
        prt_sb = sb.tile([1, 4], F32)
        g8_sb = sb.tile([NCORES, 4], F32)
        cmb_sb = sb.tile([1, 4], F32)
        logit_sb = sb.tile([1, NCAT], F32)

        # ---------- phase 1: m-major streamed IX = inc.T @ X ----------
        with tc.tile_pool(name="xipool", bufs=16) as xipool, \
             tc.tile_pool(name="pp1", bufs=4, space="PSUM") as pp1, \
             tc.tile_pool(name="ppw", bufs=1, space="PSUM") as ppw:

            accs = [pp1.tile([128, D], F32, tag="p1", name=f"acc{es}")
                    for es in range(ET)]

            for g in range(NG):
                xi = xipool.tile([128, MG * W_ROW], BF16, tag="xi",
                                 name=f"xi{g}")
                nc.sync.dma_start(
                    xi[:], xi_d[:, MG * W_ROW * g:MG * W_ROW * (g + 1)])
                for mt in range(MG):
                    for es in range(ET):
                        nc.tensor.matmul(
                            accs[es][:],
                            xi[:, mt * W_ROW + D + 128 * es:
                               mt * W_ROW + D + 128 * (es + 1)],
                            xi[:, mt * W_ROW:mt * W_ROW + D],
                            start=(g == 0 and mt == 0),
                            stop=(g == NG - 1 and mt == MG - 1))
                if g in (0, 10, 20):
                    # warm the collective path early: absorbs CC-engine
                    # cold start + inter-core launch skew while PE works
                    nc.gpsimd.collective_compute(
                        "AllGather", OP.bypass, replica_groups=groups,
                        ins=[wrm_d.opt()], outs=[wgt_d.opt()])
                if 6 <= g <= 14 and g % 2 == 0:
                    load_weights((g - 6) // 2)
                if g == 18:
                    # device precompute, hidden under phase 1:
                    # W2 = ec_W_proj @ fc_W ; b2 = ecb @ fcW + fcb
                    for j, (cj, oj) in enumerate(zip(DCH, DOF)):
                        w2p = ppw.tile([128, NCAT], F32, tag="w",
                                       name=f"w2p{j}")
                        for i, (ci, _) in enumerate(zip(DCH, DOF)):
                            nc.tensor.matmul(w2p[:cj, :],
                                             ecpT_sb[:ci, i, oj:oj + cj],
                                             fcw_sb[:ci, i, :],
                                             start=(i == 0), stop=(i == 2))
                        nc.scalar.copy(w2_sb[:cj, j, :], w2p[:cj, :])
                    b2p = ppw.tile([1, NCAT], F32, tag="w", name="b2p")
                    for i, (ci, _) in enumerate(zip(DCH, DOF)):
                        nc.tensor.matmul(b2p[:], ecbc_sb[:ci, i, :],
                                         fcw_sb[:ci, i, :],
                                         start=(i == 0), stop=(i == 2))
                    nc.vector.tensor_add(b2_sb[:, 0:NCAT], b2p[:],
                                         fcb_sb[:])

            # psum -> sbuf (inside pp1 scope)
            for es in range(ET):
                nc.vector.tensor_copy(ix_sb[:, es, :], accs[es][:])

        # ---------- batched tail ----------
        with tc.tile_pool(name="ppt", bufs=3, space="PSUM") as ppt, \
             tc.tile_pool(name="ppa", bufs=2, space="PSUM") as ppa, \
             tc.tile_pool(name="ppj", bufs=2, space="PSUM") as ppj, \
             tc.tile_pool(name="pps", bufs=1, space="PSUM") as pps:

            def ef_transpose(es):
                for i, (c, o) in enumerate(zip(DCH, DOF)):
                    tp = ppt.tile([128, 128], F32, tag="tp",
                                  name=f"tpe_{es}_{i}")
                    nc.tensor.transpose(tp[:c, :], ef_sb[:, es, o:o + c],
                                        ident[:])
                    nc.scalar.copy(
                        efT_sb[:c, i,
                               128 * es:128 * (es + 1)].bitcast(F32R),
                        tp[:c, :])

            # stage 1+2 per es: IX -> ixT ; att (f32r) ; softmax ; ef
            for es in range(ET):
                for i, (c, o) in enumerate(zip(DCH, DOF)):
                    tp = ppt.tile([128, 128], F32, tag="tp",
                                  name=f"tpa_{es}_{i}")
                    nc.tensor.transpose(tp[:c, :], ix_sb[:, es, o:o + c],
                                        ident[:])
                    nc.scalar.copy(
                        ixT_sb[:c, i,
                               128 * es:128 * (es + 1)].bitcast(F32R),
                        tp[:c, :])
                att = ppa.tile([128, D], F32, tag="att", name=f"att{es}")
                for i, (c, _) in enumerate(zip(DCH, DOF)):
                    nc.tensor.matmul(att[:],
                                     r(ixT_sb[:c, i,
                                              128 * es:128 * (es + 1)]),
                                     r(watt_sb[:c, i, :]),
                                     start=(i == 0), stop=(i == 2))
                nmax = stat_sb[:, es, 0:1]
                nc.vector.tensor_reduce(nmax, att[:], axis=AX.X, op=OP.max,
                                        negate=True)
                rsum = stat_sb[:, es, 1:2]
                nc.scalar.activation(ex_sb[:, es, :], att[:], AF.Exp,
                                     bias=nmax, scale=1.0, accum_out=rsum)
                rcp = stat_sb[:, es, 2:3]
                nc.vector.reciprocal(rcp, rsum)
                nc.vector.scalar_tensor_tensor(
                    ef_sb[:, es, :], ex_sb[:, es, :], rcp, ix_sb[:, es, :],
                    op0=OP.mult, op1=OP.mult)
                if es >= 1:
                    ef_transpose(es - 1)
            ef_transpose(ET - 1)

            # stage 4: ef2T = alpha*eftT + (1-alpha) * (W_proj.T @ efT)
            for j, (cj, oj) in enumerate(zip(DCH, DOF)):
                prj = ppj.tile([128, E_SH], F32, tag="prj", name=f"prj{j}")
                for i, (ci, _) in enumerate(zip(DCH, DOF)):
                    nc.tensor.matmul(prj[:cj, :],
                                     r(wproj_sb[:ci, i, oj:oj + cj]),
                                     r(efT_sb[:ci, i, :]),
                                     start=(i == 0), stop=(i == 2))
                nc.vector.scalar_tensor_tensor(
                    ef2T_sb[:cj, j, :].bitcast(F32R), prj[:cj, :],
                    float(1.0 - alpha), efs_sb[:cj, j, :], op0=OP.mult,
                    op1=OP.add)

            # stage 5: scores first on PE, then G2 while exp runs
            sc = ppj.tile([1, E_SH], F32, tag="prj", name="sc")
            for i, (ci, _) in enumerate(zip(DCH, DOF)):
                nc.tensor.matmul(sc[:], r(ecwatt_sb[:ci, i, :]),
                                 r(ef2T_sb[:ci, i, :]),
                                 start=(i == 0), stop=(i == 2))
            expw = expw_sb[:, 0:E_SH]
            z = expw_sb[:, E_SH:E_SH + 1]
            nc.scalar.activation(expw, sc[:], AF.Exp, bias=0.0, scale=1.0,
                                 accum_out=z)

            # stage 6: G2 = ef2 @ W2 (PE, overlaps exp) ; p2 = expw.T @ G2
            for es in range(ET):
                g2 = pps.tile([128, NCAT], F32, tag="small", name=f"g2_{es}")
                for i, (ci, _) in enumerate(zip(DCH, DOF)):
                    nc.tensor.matmul(g2[:],
                                     ef2T_sb[:ci, i,
                                             128 * es:128 * (es + 1)],
                                     w2_sb[:ci, i, :],
                                     start=(i == 0), stop=(i == 2))
                nc.scalar.copy(g2_sb[:, es, :], g2[:])
            for es in range(ET):
                tc1 = ppt.tile([128, 128], F32, tag="tp", name=f"tc1_{es}")
                nc.tensor.transpose(tc1[:, 0:1],
                                    expw[0:1, 128 * es:128 * (es + 1)],
                                    ident[0:1, 0:1])
                nc.scalar.copy(expcol_sb[:, es:es + 1], tc1[:, 0:1])
            p2 = pps.tile([1, NCAT], F32, tag="small", name="p2")
            for es in range(ET):
                nc.tensor.matmul(p2[:], expcol_sb[:, es:es + 1],
                                 g2_sb[:, es, :], start=(es == 0),
                                 stop=(es == ET - 1))

            nc.scalar.copy(prt_sb[:, 0:NCAT], p2[:])
            nc.scalar.copy(prt_sb[:, NCAT:NCAT + 1], z)
            nc.sync.dma_start(prt_d[:], prt_sb[0:1, :])

            # ---------- AllGather + tiny epilogue ----------
            nc.gpsimd.collective_compute(
                "AllGather", OP.bypass, replica_groups=groups,
                ins=[prt_d.opt()], outs=[gat_d.opt()])
            nc.sync.dma_start(g8_sb[:],
                              gat_d[:].rearrange("(c k) -> c k", c=NCORES))
            cmb = pps.tile([1, 4], F32, tag="small", name="cmb")
            nc.tensor.matmul(cmb[:], ones8_sb[:], g8_sb[:], start=True,
                             stop=True)
            rz = expw_sb[:, E_SH + 1:E_SH + 2]
            nc.vector.reciprocal(rz, cmb[:, NCAT:NCAT + 1])
            nc.vector.scalar_tensor_tensor(
                logit_sb[:], cmb[:, 0:NCAT], rz, b2_sb[:, 0:NCAT],
                op0=OP.mult, op1=OP.add)
            nc.sync.dma_start(out_d[:], logit_sb[:])

    nc.compile()
    return nc


_CACHE = {}


def get_nc(alpha: float):
    if alpha not in _CACHE:
        _CACHE[alpha] = _build(alpha)
    return _CACHE[alpha]


def _pack(a2d, rows, width):
    # (rows*128, width) row-major -> (128, rows*width) partition-major
    return np.ascontiguousarray(
        a2d.reshape(rows, 128, width).transpose(1, 0, 2).reshape(
            128, rows * width))


def make_in_maps(node_feats, edge_feats, inc_mat, W_att, W_proj,
                 ec_W_att, ec_W_proj, ec_b_proj, fc_W, fc_b):
    import ml_dtypes
    cc = lambda a: np.ascontiguousarray(np.asarray(a, np.float32))
    x_bf = np.asarray(node_feats, np.float32).astype(ml_dtypes.bfloat16)
    inc_f = np.asarray(inc_mat, np.float32)
    eft = np.asarray(edge_feats, np.float32).T  # (D, E)
    common = dict(watt=cc(W_att), wproj=cc(W_proj),
                  ecwatt=cc(ec_W_att).reshape(D, 1),
                  ecpt=cc(np.asarray(ec_W_proj, np.float32).T),
                  ecb=cc(ec_b_proj), fcw=cc(fc_W), fcb=cc(fc_b))
    in_maps = []
    for c in range(NCORES):
        sl = slice(E_SH * c, E_SH * (c + 1))
        inc_bf = inc_f[:, sl].astype(ml_dtypes.bfloat16)
        # interleave per m-row: [x_row (300) | inc_row (512)] bf16
        xi = np.empty((M, D + E_SH), dtype=ml_dtypes.bfloat16)
        xi[:, 0:D] = x_bf
        xi[:, D:] = inc_bf
        in_maps.append(dict(
            xi=_pack(xi, MT, D + E_SH),
            eft=np.ascontiguousarray(eft[:, sl]),
            **common))
    return in_maps


def kernel(node_feats, edge_feats, inc_mat, W_att, W_proj, alpha,
           ec_W_att, ec_W_proj, ec_b_proj, fc_W, fc_b, trace=False,
           mode=None):
    nc = get_nc(float(np.asarray(alpha)))
    in_maps = make_in_maps(node_feats, edge_feats, inc_mat, W_att, W_proj,
                           ec_W_att, ec_W_proj, ec_b_proj, fc_W, fc_b)
    res = run_bass_kernel_spmd(nc, in_maps, list(range(NCORES)), trace=trace)
    kernel.last_results = res
    return res.results[0]["out"].reshape(NCAT).astype(np.float32)
